# revision 1
# baseline (speedup 1.0000x reference)
"""KAN-FNO block on 8 Trainium2 NeuronCores.

Strategy (per sharding hint): data-parallel over batch (16 -> 2 per core),
weights replicated. The rfft2/irfft2 with 16x16 kept modes is implemented as
small dense DFT matmuls (only 32 h-freqs x 16 w-freqs are ever used), so the
whole block lowers to matmuls + elementwise ops that XLA-Neuron supports.
"""
import numpy as np
import jax
import jax.numpy as jnp
from functools import partial

GRID_SIZE = 5
SPLINE_ORDER = 3
MODES = 16
H = W = 128
C = 64
B = 16
NCORES = 8

HI = jax.lax.Precision.HIGHEST


def _dft_consts():
    # forward: rows kept R = [0..15] + [112..127]; cols 0..15
    r = np.concatenate([np.arange(MODES), np.arange(H - MODES, H)]).astype(np.float64)
    h = np.arange(H, dtype=np.float64)
    th = 2.0 * np.pi * np.outer(r, h) / H          # (32, 128)
    Ah_c, Ah_s = np.cos(th), np.sin(th)
    w = np.arange(W, dtype=np.float64)
    c = np.arange(MODES, dtype=np.float64)
    tw = 2.0 * np.pi * np.outer(w, c) / W          # (128, 16)
    Fw_c, Fw_s = np.cos(tw), np.sin(tw)
    # inverse over h: exp(+2*pi*i*r*h'/H)
    # inverse over w: doubling for c>=1, real part only
    g = np.ones(MODES); g[1:] = 2.0
    scale = 1.0 / (H * W)
    Ew_c = (np.cos(tw) * g[None, :]).T * scale     # (16, 128)
    Ew_s = (np.sin(tw) * g[None, :]).T * scale     # (16, 128)
    f32 = lambda a: jnp.asarray(a, dtype=jnp.float32)
    return (f32(Ah_c), f32(Ah_s), f32(Fw_c), f32(Fw_s), f32(Ew_c), f32(Ew_s))


def _make_grid():
    hh = 2.0 / GRID_SIZE
    return jnp.arange(-SPLINE_ORDER, GRID_SIZE + SPLINE_ORDER + 1,
                      dtype=jnp.float32) * hh - 1.0


def _b_splines(x, grid):
    xe = x[..., None]
    bases = ((xe >= grid[:-1]) & (xe < grid[1:])).astype(x.dtype)
    for k in range(1, SPLINE_ORDER + 1):
        left = (xe - grid[:-(k + 1)]) / (grid[k:-1] - grid[:-(k + 1)])
        right = (grid[k + 1:] - xe) / (grid[k + 1:] - grid[1:-k])
        bases = left * bases[..., :-1] + right * bases[..., 1:]
    return bases


def _kan_linear(x, base_w, spline_mat, grid):
    base = jnp.dot(jax.nn.silu(x), base_w.T, precision=HI)
    b = _b_splines(x, grid)                         # (N, C, K)
    n = x.shape[0]
    spline = jnp.dot(b.reshape(n, -1), spline_mat, precision=HI)
    return base + spline


def _block(x, w1r, w1i, w2r, w2i, conv_w, conv_b, k1b, k1s, k2b, k2s, consts):
    # x: (b_loc, C, H, W)
    Ah_c, Ah_s, Fw_c, Fw_s, Ew_c, Ew_s = consts
    grid = _make_grid()
    # ---- forward truncated DFT ----
    Tr = jnp.einsum('bchw,wk->bchk', x, Fw_c, precision=HI)
    Ti = -jnp.einsum('bchw,wk->bchk', x, Fw_s, precision=HI)
    Xr = jnp.einsum('rh,bchk->bcrk', Ah_c, Tr, precision=HI) \
       + jnp.einsum('rh,bchk->bcrk', Ah_s, Ti, precision=HI)
    Xi = jnp.einsum('rh,bchk->bcrk', Ah_c, Ti, precision=HI) \
       - jnp.einsum('rh,bchk->bcrk', Ah_s, Tr, precision=HI)
    # ---- per-frequency channel mix (w1 on rows 0..15, w2 on rows 112..127) ----
    wr = jnp.concatenate([w1r, w2r], axis=2)        # (C, C, 32, 16)
    wi = jnp.concatenate([w1i, w2i], axis=2)
    Yr = jnp.einsum('birk,iork->bork', Xr, wr, precision=HI) \
       - jnp.einsum('birk,iork->bork', Xi, wi, precision=HI)
    Yi = jnp.einsum('birk,iork->bork', Xr, wi, precision=HI) \
       + jnp.einsum('birk,iork->bork', Xi, wr, precision=HI)
    # ---- inverse: over h' (exp(+i th)), then real irfft over w ----
    Zr = jnp.einsum('rh,bork->bohk', Ah_c, Yr, precision=HI) \
       - jnp.einsum('rh,bork->bohk', Ah_s, Yi, precision=HI)
    Zi = jnp.einsum('rh,bork->bohk', Ah_c, Yi, precision=HI) \
       + jnp.einsum('rh,bork->bohk', Ah_s, Yr, precision=HI)
    x1 = jnp.einsum('bohk,kw->bohw', Zr, Ew_c, precision=HI) \
       - jnp.einsum('bohk,kw->bohw', Zi, Ew_s, precision=HI)
    # ---- 1x1 conv ----
    x2 = jnp.einsum('bchw,oc->bohw', x, conv_w, precision=HI) \
       + conv_b[None, :, None, None]
    y = x1 + x2
    bl = y.shape[0]
    y_flat = y.transpose(0, 2, 3, 1).reshape(-1, C)
    y_flat = _kan_linear(y_flat, k1b, k1s, grid)
    y_flat = _kan_linear(y_flat, k2b, k2s, grid)
    y = y_flat.reshape(bl, H, W, C).transpose(0, 3, 1, 2)
    return jax.nn.gelu(y, approximate=False)


_CONSTS = None
_FN = None


def _get_fn():
    global _CONSTS, _FN
    if _FN is None:
        _CONSTS = _dft_consts()
        consts = _CONSTS

        def run(x, w1r, w1i, w2r, w2i, cw, cb, k1b, k1s, k2b, k2s):
            return _block(x, w1r, w1i, w2r, w2i, cw, cb, k1b, k1s, k2b, k2s,
                          consts)

        _FN = jax.pmap(run, in_axes=(0,) + (None,) * 10, devices=jax.devices()[:NCORES])
    return _FN


def kernel(x, spec_w1_r, spec_w1_i, spec_w2_r, spec_w2_i, conv_w, conv_b,
           k1_base, k1_spline, k1_scaler, k2_base, k2_spline, k2_scaler):
    fn = _get_fn()
    # host-side weight prep: fold scaler into spline weights, reshape to matmul
    k1s = (k1_spline * k1_scaler[..., None])        # (o, i, K)
    k2s = (k2_spline * k2_scaler[..., None])
    K = GRID_SIZE + SPLINE_ORDER
    k1s_mat = np.transpose(k1s, (1, 2, 0)).reshape(C * K, C).astype(np.float32)
    k2s_mat = np.transpose(k2s, (1, 2, 0)).reshape(C * K, C).astype(np.float32)
    xs = np.asarray(x, dtype=np.float32).reshape(NCORES, B // NCORES, C, H, W)
    out = fn(jnp.asarray(xs), jnp.asarray(spec_w1_r), jnp.asarray(spec_w1_i),
             jnp.asarray(spec_w2_r), jnp.asarray(spec_w2_i),
             jnp.asarray(conv_w), jnp.asarray(conv_b),
             jnp.asarray(k1_base), jnp.asarray(k1s_mat),
             jnp.asarray(k2_base), jnp.asarray(k2s_mat))
    return np.asarray(out).reshape(B, C, H, W)


# revision 3
# speedup vs baseline: 1.0443x; 1.0443x over previous
"""KAN-FNO block on 8 Trainium2 NeuronCores.

Strategy (per sharding hint): data-parallel over batch (16 -> 2 per core),
weights replicated. The rfft2/irfft2 with 16x16 kept modes is implemented as
small dense DFT matmuls (only 32 h-freqs x 16 w-freqs are ever used), so the
whole block lowers to matmuls + elementwise ops that XLA-Neuron supports.
"""
import numpy as np
import jax
import jax.numpy as jnp
from functools import partial

GRID_SIZE = 5
SPLINE_ORDER = 3
MODES = 16
H = W = 128
C = 64
B = 16
NCORES = 8

HI = jax.lax.Precision.HIGHEST


def _dft_consts():
    # forward: rows kept R = [0..15] + [112..127]; cols 0..15
    r = np.concatenate([np.arange(MODES), np.arange(H - MODES, H)]).astype(np.float64)
    h = np.arange(H, dtype=np.float64)
    th = 2.0 * np.pi * np.outer(r, h) / H          # (32, 128)
    Ah_c, Ah_s = np.cos(th), np.sin(th)
    w = np.arange(W, dtype=np.float64)
    c = np.arange(MODES, dtype=np.float64)
    tw = 2.0 * np.pi * np.outer(w, c) / W          # (128, 16)
    Fw_c, Fw_s = np.cos(tw), np.sin(tw)
    # inverse over h: exp(+2*pi*i*r*h'/H)
    # inverse over w: doubling for c>=1, real part only
    g = np.ones(MODES); g[1:] = 2.0
    scale = 1.0 / (H * W)
    Ew_c = (np.cos(tw) * g[None, :]).T * scale     # (16, 128)
    Ew_s = (np.sin(tw) * g[None, :]).T * scale     # (16, 128)
    f32 = lambda a: jnp.asarray(a, dtype=jnp.float32)
    return (f32(Ah_c), f32(Ah_s), f32(Fw_c), f32(Fw_s), f32(Ew_c), f32(Ew_s))


def _make_grid():
    hh = 2.0 / GRID_SIZE
    return jnp.arange(-SPLINE_ORDER, GRID_SIZE + SPLINE_ORDER + 1,
                      dtype=jnp.float32) * hh - 1.0


def _b_splines(x, grid):
    xe = x[..., None]
    bases = ((xe >= grid[:-1]) & (xe < grid[1:])).astype(x.dtype)
    for k in range(1, SPLINE_ORDER + 1):
        left = (xe - grid[:-(k + 1)]) / (grid[k:-1] - grid[:-(k + 1)])
        right = (grid[k + 1:] - xe) / (grid[k + 1:] - grid[1:-k])
        bases = left * bases[..., :-1] + right * bases[..., 1:]
    return bases


def _kan_linear(x, base_w, spline_mat, grid):
    base = jnp.dot(jax.nn.silu(x), base_w.T, precision=HI)
    b = _b_splines(x, grid)                         # (N, C, K)
    n = x.shape[0]
    spline = jnp.dot(b.reshape(n, -1), spline_mat, precision=HI)
    return base + spline


def _block(x, w1r, w1i, w2r, w2i, conv_w, conv_b, k1b, k1s, k2b, k2s, consts):
    # x: (b_loc, C, H, W)
    Ah_c, Ah_s, Fw_c, Fw_s, Ew_c, Ew_s = consts
    grid = _make_grid()
    # ---- forward truncated DFT ----
    Tr = jnp.einsum('bchw,wk->bchk', x, Fw_c, precision=HI)
    Ti = -jnp.einsum('bchw,wk->bchk', x, Fw_s, precision=HI)
    Xr = jnp.einsum('rh,bchk->bcrk', Ah_c, Tr, precision=HI) \
       + jnp.einsum('rh,bchk->bcrk', Ah_s, Ti, precision=HI)
    Xi = jnp.einsum('rh,bchk->bcrk', Ah_c, Ti, precision=HI) \
       - jnp.einsum('rh,bchk->bcrk', Ah_s, Tr, precision=HI)
    # ---- per-frequency channel mix (w1 on rows 0..15, w2 on rows 112..127) ----
    wr = jnp.concatenate([w1r, w2r], axis=2)        # (C, C, 32, 16)
    wi = jnp.concatenate([w1i, w2i], axis=2)
    Yr = jnp.einsum('birk,iork->bork', Xr, wr, precision=HI) \
       - jnp.einsum('birk,iork->bork', Xi, wi, precision=HI)
    Yi = jnp.einsum('birk,iork->bork', Xr, wi, precision=HI) \
       + jnp.einsum('birk,iork->bork', Xi, wr, precision=HI)
    # ---- inverse: over h' (exp(+i th)), then real irfft over w ----
    Zr = jnp.einsum('rh,bork->bohk', Ah_c, Yr, precision=HI) \
       - jnp.einsum('rh,bork->bohk', Ah_s, Yi, precision=HI)
    Zi = jnp.einsum('rh,bork->bohk', Ah_c, Yi, precision=HI) \
       + jnp.einsum('rh,bork->bohk', Ah_s, Yr, precision=HI)
    x1 = jnp.einsum('bohk,kw->bohw', Zr, Ew_c, precision=HI) \
       - jnp.einsum('bohk,kw->bohw', Zi, Ew_s, precision=HI)
    # ---- 1x1 conv ----
    x2 = jnp.einsum('bchw,oc->bohw', x, conv_w, precision=HI) \
       + conv_b[None, :, None, None]
    y = x1 + x2
    bl = y.shape[0]
    y_flat = y.transpose(0, 2, 3, 1).reshape(-1, C)
    y_flat = _kan_linear(y_flat, k1b, k1s, grid)
    y_flat = _kan_linear(y_flat, k2b, k2s, grid)
    y = y_flat.reshape(bl, H, W, C).transpose(0, 3, 1, 2)
    return jax.nn.gelu(y, approximate=False)


_CONSTS = None
_FN = None


def _get_fn():
    global _CONSTS, _FN
    if _FN is None:
        _CONSTS = _dft_consts()
        consts = _CONSTS

        def run(x, w1r, w1i, w2r, w2i, cw, cb, k1b, k1s, k2b, k2s):
            return _block(x, w1r, w1i, w2r, w2i, cw, cb, k1b, k1s, k2b, k2s,
                          consts)

        _FN = jax.pmap(run, in_axes=(0,) + (None,) * 10, devices=jax.devices()[:NCORES])
    return _FN


def kernel(x, spec_w1_r, spec_w1_i, spec_w2_r, spec_w2_i, conv_w, conv_b,
           k1_base, k1_spline, k1_scaler, k2_base, k2_spline, k2_scaler):
    fn = _get_fn()
    # host-side weight prep: fold scaler into spline weights, reshape to matmul
    k1s = (k1_spline * k1_scaler[..., None])        # (o, i, K)
    k2s = (k2_spline * k2_scaler[..., None])
    K = GRID_SIZE + SPLINE_ORDER
    k1s_mat = np.transpose(k1s, (1, 2, 0)).reshape(C * K, C).astype(np.float32)
    k2s_mat = np.transpose(k2s, (1, 2, 0)).reshape(C * K, C).astype(np.float32)
    xs = np.asarray(x, dtype=np.float32).reshape(NCORES, B // NCORES, C, H, W)
    out = fn(jnp.asarray(xs), jnp.asarray(spec_w1_r), jnp.asarray(spec_w1_i),
             jnp.asarray(spec_w2_r), jnp.asarray(spec_w2_i),
             jnp.asarray(conv_w), jnp.asarray(conv_b),
             jnp.asarray(k1_base), jnp.asarray(k1s_mat),
             jnp.asarray(k2_base), jnp.asarray(k2s_mat))
    return np.asarray(out).reshape(B, C, H, W)
